# revision 23
# baseline (speedup 1.0000x reference)
"""Adaptive graph construction kernel for 8 TRN2 NeuronCores.

Reference computation (per batch item n of 128):
    qa = Wa @ x[n] + ba        # [21, 256*25]   (x[n] viewed as [64, 256*25])
    kb = Wb @ x[n] + bb        # [21, 256*25]
    B[v, w] = sum_{i, t} qa[i, (t, v)] * kb[i, (t, w)]     # [25, 25]
    out[n]  = softmax(A + tanh(B / (21*256)) * alpha, axis=-1)

Strategy: pure data parallelism - batch N=128 sharded across 8 cores, no
collectives. Per core, batches processed in pairs:
  1. Projection: both batches' x stacked on 128 partitions, one GEMM per
     512-chunk with a 4-block stationary [128, 128] -> psum partitions
     0-20: qa even, 32-52: kb even, 64-84: qa odd, 96-116: kb odd.
     Bias-add + bf16 cast fused into the PSUM->SBUF copy (alternating
     ScalarE/VectorE so neither engine bottlenecks).
  2. Per-chunk row-class swap DMAs put kb at qa's base partitions (the PE
     needs equal operand bases); chunk granularity keeps them off the
     critical path.
  3. Gram: PE matmul overhead is ~60 cycles regardless of free size, so
     each matmul streams 50 columns = 2 consecutive timesteps:
     [K=21, M=50, N=50]; the two diagonal 25x25 blocks of the output are
     the useful products, off-diagonal blocks accumulate junk harmlessly.
     128 matmuls per batch accumulate into one [50, 50] PSUM tile.
     The issue order is software-pipelined: pair k+1's projection is
     issued before pair k's gram so the in-order PE never stalls on the
     swap DMAs (stalls > ~5us would also re-throttle the PE clock).
  4. Extraction: ACT-copy [50,50] to SBUF bf16; two accumulating matmuls
     with block-selector stationaries [I;0], [0;I] fold the diagonal
     blocks exactly. Epilogue: tanh/alpha/A/softmax. Compute bf16, PSUM
     f32; tolerance 2e-2.
"""

import numpy as np
import ml_dtypes

N, C, T, V = 128, 64, 256, 25
IC = 21
NCORES = 8
NB = N // NCORES          # 16 batches per core
NPAIR = NB // 2           # 8
TV = T * V                # 6400
CHUNK = 512
NCHUNK = (TV + CHUNK - 1) // CHUNK
NTP = T // 2              # 128 gram matmuls per batch (2 t's each)


def _build_bass():
    import concourse.bass as bass
    import concourse.tile as tile
    from concourse import bacc, mybir

    f32 = mybir.dt.float32
    bf16 = mybir.dt.bfloat16

    nc = bacc.Bacc("TRN2", target_bir_lowering=False, debug=False,
                   num_devices=NCORES)

    x_ext = nc.dram_tensor("x", [NB, C, TV], bf16, kind="ExternalInput")
    wqk_ext = nc.dram_tensor("wqk", [2 * C, 128], bf16, kind="ExternalInput")
    bias_ext = nc.dram_tensor("bias", [128, 1], f32, kind="ExternalInput")
    emat_ext = nc.dram_tensor("emat", [2 * V, 2 * V], bf16,
                              kind="ExternalInput")
    a_ext = nc.dram_tensor("amat", [V, V], f32, kind="ExternalInput")
    alpha_ext = nc.dram_tensor("alpha", [1, 1], f32, kind="ExternalInput")
    out_ext = nc.dram_tensor("out", [NB, V, V], f32, kind="ExternalOutput")

    with tile.TileContext(nc) as tc:
        with (
            tc.tile_pool(name="singles", bufs=1) as singles,
            tc.tile_pool(name="xb", bufs=3) as xb_pool,
            tc.tile_pool(name="yz", bufs=3) as yz_pool,
            tc.tile_pool(name="sh", bufs=3) as sh_pool,
            tc.tile_pool(name="gs", bufs=3) as gs_pool,
            tc.tile_pool(name="zp", bufs=4, space="PSUM") as zp_pool,
            tc.tile_pool(name="gp", bufs=2, space="PSUM") as gp_pool,
            tc.tile_pool(name="o2", bufs=2, space="PSUM") as o2_pool,
            tc.tile_pool(name="ep", bufs=3) as ep_pool,
        ):
            wqk = singles.tile([2 * C, 128], bf16)
            nc.sync.dma_start(out=wqk[:], in_=wqk_ext[:, :])
            bias = singles.tile([128, 1], f32)
            nc.sync.dma_start(out=bias[:], in_=bias_ext[:, :])
            emat = singles.tile([2 * V, 2 * V], bf16)
            nc.sync.dma_start(out=emat[:], in_=emat_ext[:, :])
            amat = singles.tile([V, V], f32)
            nc.sync.dma_start(out=amat[:], in_=a_ext[:, :])
            alpha_sb = singles.tile([V, 1], f32)
            nc.sync.dma_start(
                out=alpha_sb[:],
                in_=bass.AP(tensor=alpha_ext, offset=0, ap=[[0, V], [1, 1]]),
            )

            def project(pair):
                """x load + projection GEMMs + bias copies + kb swaps."""
                n0, n1 = 2 * pair, 2 * pair + 1
                xb = xb_pool.tile([128, TV], bf16)
                nc.sync.dma_start(out=xb[0:C, :], in_=x_ext[n0])
                nc.gpsimd.dma_start(out=xb[C:128, :], in_=x_ext[n1])
                yz = yz_pool.tile([128, TV], bf16)
                sh = sh_pool.tile([128, TV], bf16)
                for ck in range(NCHUNK):
                    c0 = ck * CHUNK
                    cw = min(CHUNK, TV - c0)
                    zp = zp_pool.tile([128, CHUNK], f32)
                    nc.tensor.matmul(
                        out=zp[:, 0:cw],
                        lhsT=wqk[:, :],
                        rhs=xb[:, c0:c0 + cw],
                        start=True, stop=True,
                    )
                    if ck % 2 == 0:
                        nc.scalar.activation(
                            out=yz[:, c0:c0 + cw],
                            in_=zp[:, 0:cw],
                            func=mybir.ActivationFunctionType.Identity,
                            bias=bias[:],
                            scale=1.0,
                        )
                    else:
                        nc.vector.tensor_scalar(
                            out=yz[:, c0:c0 + cw],
                            in0=zp[:, 0:cw],
                            scalar1=bias[:, 0:1], scalar2=None,
                            op0=mybir.AluOpType.add,
                        )
                    nc.sync.dma_start(
                        out=sh[0:IC, c0:c0 + cw],
                        in_=yz[32:32 + IC, c0:c0 + cw])
                    nc.gpsimd.dma_start(
                        out=sh[64:64 + IC, c0:c0 + cw],
                        in_=yz[96:96 + IC, c0:c0 + cw])
                return yz, sh

            def gram_and_epilogue(pair, yz, sh):
                n0, n1 = 2 * pair, 2 * pair + 1
                for h, n in ((0, n0), (1, n1)):
                    rb = 64 * h
                    gp = gp_pool.tile([2 * V, 2 * V], f32)
                    for tp in range(NTP):
                        f0 = tp * 2 * V
                        nc.tensor.matmul(
                            out=gp[:],
                            lhsT=yz[rb:rb + IC, f0:f0 + 2 * V],
                            rhs=sh[rb:rb + IC, f0:f0 + 2 * V],
                            start=(tp == 0), stop=(tp == NTP - 1),
                        )
                    gs = gs_pool.tile([2 * V, 2 * V], bf16)
                    nc.scalar.activation(
                        out=gs[:], in_=gp[:],
                        func=mybir.ActivationFunctionType.Copy,
                    )
                    o2 = o2_pool.tile([V, V], f32)
                    nc.tensor.matmul(
                        out=o2[:], lhsT=emat[:, 0:V], rhs=gs[:, 0:V],
                        start=True, stop=False, skip_group_check=True,
                    )
                    nc.tensor.matmul(
                        out=o2[:], lhsT=emat[:, V:2 * V], rhs=gs[:, V:2 * V],
                        start=False, stop=True, skip_group_check=True,
                    )
                    th = ep_pool.tile([V, V], f32, tag="th")
                    nc.scalar.activation(
                        out=th[:], in_=o2[:],
                        func=mybir.ActivationFunctionType.Tanh,
                        scale=1.0 / float(IC * T),
                    )
                    sm = ep_pool.tile([V, V], f32, tag="sm")
                    nc.vector.scalar_tensor_tensor(
                        out=sm[:], in0=th[:], scalar=alpha_sb[:, 0:1],
                        in1=amat[:],
                        op0=mybir.AluOpType.mult, op1=mybir.AluOpType.add,
                    )
                    negmax = ep_pool.tile([V, 1], f32, tag="negmax")
                    nc.vector.tensor_reduce(
                        out=negmax[:], in_=sm[:],
                        axis=mybir.AxisListType.X, op=mybir.AluOpType.max,
                        negate=True,
                    )
                    esum = ep_pool.tile([V, 1], f32, tag="esum")
                    ex = ep_pool.tile([V, V], f32, tag="ex")
                    nc.scalar.activation(
                        out=ex[:], in_=sm[:],
                        func=mybir.ActivationFunctionType.Exp,
                        bias=negmax[:, 0:1], scale=1.0,
                        accum_out=esum[:, 0:1],
                    )
                    rcp = ep_pool.tile([V, 1], f32, tag="rcp")
                    nc.vector.reciprocal(out=rcp[:], in_=esum[:])
                    res = ep_pool.tile([V, V], f32, tag="res")
                    nc.vector.tensor_scalar(
                        out=res[:], in0=ex[:], scalar1=rcp[:, 0:1],
                        scalar2=None, op0=mybir.AluOpType.mult,
                    )
                    nc.gpsimd.dma_start(out=out_ext[n], in_=res[:])

            # Software pipeline: issue pair k+1's projection before pair
            # k's gram, so the in-order PE has work while pair k's swap
            # DMAs complete.
            staged = project(0)
            for pair in range(NPAIR):
                nxt = project(pair + 1) if pair + 1 < NPAIR else None
                gram_and_epilogue(pair, *staged)
                staged = nxt

    nc.compile()
    return nc


_NC_CACHE = None


def _make_in_maps(inputs):
    x = np.asarray(inputs["x"], dtype=np.float32)
    Wa = np.asarray(inputs["Wa"], dtype=np.float32)
    Wb = np.asarray(inputs["Wb"], dtype=np.float32)
    ba = np.asarray(inputs["ba"], dtype=np.float32)
    bb = np.asarray(inputs["bb"], dtype=np.float32)
    A = np.asarray(inputs["A"], dtype=np.float32)
    alpha = np.asarray(inputs["alpha"], dtype=np.float32)

    # Stationary [128, 128]: col j -> psum partition j of the projection.
    wqk = np.zeros((2 * C, 128), dtype=np.float32)
    wqk[0:C, 0:IC] = Wa.T
    wqk[0:C, 32:32 + IC] = Wb.T
    wqk[C:2 * C, 64:64 + IC] = Wa.T
    wqk[C:2 * C, 96:96 + IC] = Wb.T
    wqk = wqk.astype(ml_dtypes.bfloat16)

    bias = np.zeros((128, 1), dtype=np.float32)
    bias[0:IC, 0] = ba
    bias[32:32 + IC, 0] = bb
    bias[64:64 + IC, 0] = ba
    bias[96:96 + IC, 0] = bb

    # emat cols 0-24: [I; 0] selects the t0 diag rows; cols 25-49: [0; I]
    emat = np.zeros((2 * V, 2 * V), dtype=np.float32)
    emat[0:V, 0:V] = np.eye(V)
    emat[V:2 * V, V:2 * V] = np.eye(V)
    emat = emat.astype(ml_dtypes.bfloat16)

    amat = np.ascontiguousarray(A.reshape(V, V))
    alpha_in = np.ascontiguousarray(alpha.reshape(1, 1))

    xbf = np.ascontiguousarray(x.reshape(NCORES, NB, C, TV)).astype(
        ml_dtypes.bfloat16)
    return [
        {"x": xbf[i], "wqk": wqk, "bias": bias, "emat": emat,
         "amat": amat, "alpha": alpha_in}
        for i in range(NCORES)
    ]


def kernel(x, Wa, ba, Wb, bb, A, alpha):
    from concourse.bass_utils import run_bass_kernel_spmd

    global _NC_CACHE
    if _NC_CACHE is None:
        _NC_CACHE = _build_bass()
    nc = _NC_CACHE

    in_maps = _make_in_maps(dict(x=x, Wa=Wa, ba=ba, Wb=Wb, bb=bb, A=A,
                                 alpha=alpha))
    res = run_bass_kernel_spmd(nc, in_maps, core_ids=list(range(NCORES)))
    out = np.concatenate([r["out"] for r in res.results], axis=0)
    return out.reshape(N, V, V)


# revision 33
# speedup vs baseline: 1.2817x; 1.2817x over previous
"""Adaptive graph construction kernel for 8 TRN2 NeuronCores.

Reference computation (per batch item n of 128):
    qa = Wa @ x[n] + ba        # [21, 256*25]   (x[n] viewed as [64, 256*25])
    kb = Wb @ x[n] + bb        # [21, 256*25]
    B[v, w] = sum_{i, t} qa[i, (t, v)] * kb[i, (t, w)]     # [25, 25]
    out[n]  = softmax(A + tanh(B / (21*256)) * alpha, axis=-1)

Strategy: pure data parallelism - batch N=128 sharded across 8 cores, no
collectives. Per core, batches processed in pairs:
  1. Projection: both batches' x stacked on 128 partitions, one GEMM per
     512-chunk with a 4-block stationary [128, 128] -> psum partitions
     0-20: qa even, 32-52: kb even, 64-84: qa odd, 96-116: kb odd.
     Bias-add + bf16 cast fused into the PSUM->SBUF copy (alternating
     ScalarE/VectorE so neither engine bottlenecks).
  2. Row-class swap DMAs (per 2 chunks) put kb at qa's base partitions
     (the PE needs equal operand bases); chunk granularity keeps them off
     the critical path.
  3. Gram: PE matmul overhead is ~60 cycles regardless of free size, so
     each matmul streams 50 columns = 2 consecutive timesteps:
     [K=21, M=50, N=50]; the two diagonal 25x25 blocks of the output are
     the useful products, off-diagonal blocks accumulate junk harmlessly.
     128 matmuls per batch accumulate into one [50, 50] PSUM tile.
     The emission is software-pipelined at matmul granularity: pair k+1's
     projection matmuls are interleaved into pair k's gram chain (one per
     ~19 quads) so the in-order PE sees a single dense stream - no stalls
     on the swap DMAs and no HAM clock re-throttle gaps.
  4. Extraction: ACT-copy [50,50] to SBUF bf16; two accumulating matmuls
     with block-selector stationaries [I;0], [0;I] fold the diagonal
     blocks exactly. Epilogue: tanh/alpha/A/softmax. Compute bf16, PSUM
     f32; tolerance 2e-2.
"""

import numpy as np
import ml_dtypes

N, C, T, V = 128, 64, 256, 25
IC = 21
NCORES = 8
NB = N // NCORES          # 16 batches per core
NPAIR = NB // 2           # 8
TV = T * V                # 6400
CHUNK = 512
NCHUNK = (TV + CHUNK - 1) // CHUNK
NTP = T // 2              # 128 gram matmuls per batch (2 t's each)


def _build_bass():
    import concourse.bass as bass
    import concourse.tile as tile
    from concourse import bacc, mybir


    f32 = mybir.dt.float32
    bf16 = mybir.dt.bfloat16

    nc = bacc.Bacc("TRN2", target_bir_lowering=False, debug=False,
                   num_devices=NCORES)

    x_ext = nc.dram_tensor("x", [NB, C, TV], bf16, kind="ExternalInput")
    wqk_ext = nc.dram_tensor("wqk", [2 * C, 128], bf16, kind="ExternalInput")
    bias_ext = nc.dram_tensor("bias", [128, 1], f32, kind="ExternalInput")
    emat_ext = nc.dram_tensor("emat", [2 * V, 2 * V], bf16,
                              kind="ExternalInput")
    a_ext = nc.dram_tensor("amat", [V, V], f32, kind="ExternalInput")
    alpha_ext = nc.dram_tensor("alpha", [1, 1], f32, kind="ExternalInput")
    out_ext = nc.dram_tensor("out", [NB, V, V], f32, kind="ExternalOutput")

    with tile.TileContext(nc) as tc:
        with (
            tc.tile_pool(name="singles", bufs=1) as singles,
            tc.tile_pool(name="xb", bufs=3) as xb_pool,
            tc.tile_pool(name="yz", bufs=3) as yz_pool,
            tc.tile_pool(name="sh", bufs=3) as sh_pool,
            tc.tile_pool(name="gs", bufs=3) as gs_pool,
            tc.tile_pool(name="zp", bufs=4, space="PSUM") as zp_pool,
            tc.tile_pool(name="gp", bufs=2, space="PSUM") as gp_pool,
            tc.tile_pool(name="o2", bufs=2, space="PSUM") as o2_pool,
            tc.tile_pool(name="ep", bufs=3) as ep_pool,
        ):
            wqk = singles.tile([2 * C, 128], bf16)
            nc.sync.dma_start(out=wqk[:], in_=wqk_ext[:, :])
            bias = singles.tile([128, 1], f32)
            nc.sync.dma_start(out=bias[:], in_=bias_ext[:, :])
            emat = singles.tile([2 * V, 2 * V], bf16)
            nc.sync.dma_start(out=emat[:], in_=emat_ext[:, :])
            amat = singles.tile([V, V], f32)
            nc.sync.dma_start(out=amat[:], in_=a_ext[:, :])
            alpha_sb = singles.tile([V, 1], f32)
            nc.sync.dma_start(
                out=alpha_sb[:],
                in_=bass.AP(tensor=alpha_ext, offset=0, ap=[[0, V], [1, 1]]),
            )

            tiles = {}

            def project(pair):
                """x load + projection GEMMs + bias copies + kb swaps.
                Generator: yields after each PE matmul so the driver can
                interleave with the previous pair's gram chain."""
                n0, n1 = 2 * pair, 2 * pair + 1
                xb = xb_pool.tile([128, TV], bf16)
                nc.sync.dma_start(out=xb[0:C, :], in_=x_ext[n0])
                nc.gpsimd.dma_start(out=xb[C:128, :], in_=x_ext[n1])
                yz = yz_pool.tile([128, TV], bf16)
                sh = sh_pool.tile([128, TV], bf16)
                tiles[pair] = (yz, sh)
                for ck in range(NCHUNK):
                    c0 = ck * CHUNK
                    cw = min(CHUNK, TV - c0)
                    zp = zp_pool.tile([128, CHUNK], f32)
                    nc.tensor.matmul(
                        out=zp[:, 0:cw],
                        lhsT=wqk[:, :],
                        rhs=xb[:, c0:c0 + cw],
                        start=True, stop=True,
                    )
                    # split the copy across ACT and DVE: halves the
                    # latency gating the swap DMAs below
                    hw_ = cw // 2
                    nc.scalar.activation(
                        out=yz[:, c0:c0 + hw_],
                        in_=zp[:, 0:hw_],
                        func=mybir.ActivationFunctionType.Identity,
                        bias=bias[:],
                        scale=1.0,
                    )
                    nc.vector.tensor_scalar(
                        out=yz[:, c0 + hw_:c0 + cw],
                        in0=zp[:, hw_:cw],
                        scalar1=bias[:, 0:1], scalar2=None,
                        op0=mybir.AluOpType.add,
                    )
                    # swap-copy every 2 chunks: halves sequencer issues
                    if ck % 2 == 1 or ck == NCHUNK - 1:
                        s0 = (ck - 1) * CHUNK if ck % 2 == 1 else ck * CHUNK
                        send = c0 + cw
                        nc.sync.dma_start(
                            out=sh[0:IC, s0:send],
                            in_=yz[32:32 + IC, s0:send])
                        nc.gpsimd.dma_start(
                            out=sh[64:64 + IC, s0:send],
                            in_=yz[96:96 + IC, s0:send])
                    yield

            def gram_and_epilogue(pair):
                """Generator: yields after each gram PE matmul."""
                yz, sh = tiles.pop(pair)
                n0, n1 = 2 * pair, 2 * pair + 1
                for h, n in ((0, n0), (1, n1)):
                    rb = 64 * h
                    gp = gp_pool.tile([2 * V, 2 * V], f32)
                    for tp in range(NTP):
                        f0 = tp * 2 * V
                        nc.tensor.matmul(
                            out=gp[:],
                            lhsT=yz[rb:rb + IC, f0:f0 + 2 * V],
                            rhs=sh[rb:rb + IC, f0:f0 + 2 * V],
                            start=(tp == 0), stop=(tp == NTP - 1),
                            skip_group_check=True,
                        )
                        yield
                    gs = gs_pool.tile([2 * V, 2 * V], bf16)
                    nc.scalar.activation(
                        out=gs[:], in_=gp[:],
                        func=mybir.ActivationFunctionType.Copy,
                    )
                    o2 = o2_pool.tile([V, V], f32)
                    nc.tensor.matmul(
                        out=o2[:], lhsT=emat[:, 0:V], rhs=gs[:, 0:V],
                        start=True, stop=False, skip_group_check=True,
                    )
                    nc.tensor.matmul(
                        out=o2[:], lhsT=emat[:, V:2 * V], rhs=gs[:, V:2 * V],
                        start=False, stop=True, skip_group_check=True,
                    )
                    th = ep_pool.tile([V, V], f32, tag="th")
                    nc.scalar.activation(
                        out=th[:], in_=o2[:],
                        func=mybir.ActivationFunctionType.Tanh,
                        scale=1.0 / float(IC * T),
                    )
                    sm = ep_pool.tile([V, V], f32, tag="sm")
                    nc.vector.scalar_tensor_tensor(
                        out=sm[:], in0=th[:], scalar=alpha_sb[:, 0:1],
                        in1=amat[:],
                        op0=mybir.AluOpType.mult, op1=mybir.AluOpType.add,
                    )
                    negmax = ep_pool.tile([V, 1], f32, tag="negmax")
                    nc.vector.tensor_reduce(
                        out=negmax[:], in_=sm[:],
                        axis=mybir.AxisListType.X, op=mybir.AluOpType.max,
                        negate=True,
                    )
                    esum = ep_pool.tile([V, 1], f32, tag="esum")
                    ex = ep_pool.tile([V, V], f32, tag="ex")
                    nc.scalar.activation(
                        out=ex[:], in_=sm[:],
                        func=mybir.ActivationFunctionType.Exp,
                        bias=negmax[:, 0:1], scale=1.0,
                        accum_out=esum[:, 0:1],
                    )
                    rcp = ep_pool.tile([V, 1], f32, tag="rcp")
                    nc.vector.reciprocal(out=rcp[:], in_=esum[:])
                    res = ep_pool.tile([V, V], f32, tag="res")
                    nc.vector.tensor_scalar(
                        out=res[:], in0=ex[:], scalar1=rcp[:, 0:1],
                        scalar2=None, op0=mybir.AluOpType.mult,
                    )
                    nc.gpsimd.dma_start(out=out_ext[n], in_=res[:])

            # Software pipeline: pair k+1's projection matmuls are
            # interleaved INTO pair k's gram chain (1 proj per ~19 quads)
            # so the in-order PE sees one continuous dense stream - no
            # stalls on the swap DMAs and no HAM re-throttle gaps.
            for _ in project(0):
                pass
            for pair in range(NPAIR):
                pg = project(pair + 1) if pair + 1 < NPAIR else None
                stride = max(1, (2 * NTP) // (NCHUNK + 1))
                cnt = 0
                for _ in gram_and_epilogue(pair):
                    cnt += 1
                    if pg is not None and cnt % stride == 0:
                        next(pg, None)
                if pg is not None:
                    for _ in pg:
                        pass

    nc.compile()
    return nc


_NC_CACHE = None


def _make_in_maps(inputs):
    x = np.asarray(inputs["x"], dtype=np.float32)
    Wa = np.asarray(inputs["Wa"], dtype=np.float32)
    Wb = np.asarray(inputs["Wb"], dtype=np.float32)
    ba = np.asarray(inputs["ba"], dtype=np.float32)
    bb = np.asarray(inputs["bb"], dtype=np.float32)
    A = np.asarray(inputs["A"], dtype=np.float32)
    alpha = np.asarray(inputs["alpha"], dtype=np.float32)

    # Stationary [128, 128]: col j -> psum partition j of the projection.
    wqk = np.zeros((2 * C, 128), dtype=np.float32)
    wqk[0:C, 0:IC] = Wa.T
    wqk[0:C, 32:32 + IC] = Wb.T
    wqk[C:2 * C, 64:64 + IC] = Wa.T
    wqk[C:2 * C, 96:96 + IC] = Wb.T
    wqk = wqk.astype(ml_dtypes.bfloat16)

    bias = np.zeros((128, 1), dtype=np.float32)
    bias[0:IC, 0] = ba
    bias[32:32 + IC, 0] = bb
    bias[64:64 + IC, 0] = ba
    bias[96:96 + IC, 0] = bb

    # emat cols 0-24: [I; 0] selects the t0 diag rows; cols 25-49: [0; I]
    emat = np.zeros((2 * V, 2 * V), dtype=np.float32)
    emat[0:V, 0:V] = np.eye(V)
    emat[V:2 * V, V:2 * V] = np.eye(V)
    emat = emat.astype(ml_dtypes.bfloat16)

    amat = np.ascontiguousarray(A.reshape(V, V))
    alpha_in = np.ascontiguousarray(alpha.reshape(1, 1))

    xbf = np.ascontiguousarray(x.reshape(NCORES, NB, C, TV)).astype(
        ml_dtypes.bfloat16)
    return [
        {"x": xbf[i], "wqk": wqk, "bias": bias, "emat": emat,
         "amat": amat, "alpha": alpha_in}
        for i in range(NCORES)
    ]


def kernel(x, Wa, ba, Wb, bb, A, alpha):
    from concourse.bass_utils import run_bass_kernel_spmd

    global _NC_CACHE
    if _NC_CACHE is None:
        _NC_CACHE = _build_bass()
    nc = _NC_CACHE

    in_maps = _make_in_maps(dict(x=x, Wa=Wa, ba=ba, Wb=Wb, bb=bb, A=A,
                                 alpha=alpha))
    res = run_bass_kernel_spmd(nc, in_maps, core_ids=list(range(NCORES)))
    out = np.concatenate([r["out"] for r in res.results], axis=0)
    return out.reshape(N, V, V)


# revision 35
# speedup vs baseline: 1.2910x; 1.0072x over previous
"""Adaptive graph construction kernel for 8 TRN2 NeuronCores.

Reference computation (per batch item n of 128):
    qa = Wa @ x[n] + ba        # [21, 256*25]   (x[n] viewed as [64, 256*25])
    kb = Wb @ x[n] + bb        # [21, 256*25]
    B[v, w] = sum_{i, t} qa[i, (t, v)] * kb[i, (t, w)]     # [25, 25]
    out[n]  = softmax(A + tanh(B / (21*256)) * alpha, axis=-1)

Strategy: pure data parallelism - batch N=128 sharded across 8 cores, no
collectives. Per core, batches processed in pairs:
  1. Projection: both batches' x stacked on 128 partitions, one GEMM per
     512-chunk with a 4-block stationary [128, 128] -> psum partitions
     0-20: qa even, 32-52: kb even, 64-84: qa odd, 96-116: kb odd.
     Bias-add + bf16 cast fused into the PSUM->SBUF copy (alternating
     ScalarE/VectorE so neither engine bottlenecks).
  2. Row-class swap DMAs (per 2 chunks) put kb at qa's base partitions
     (the PE needs equal operand bases); chunk granularity keeps them off
     the critical path.
  3. Gram: PE matmul overhead is ~60 cycles regardless of free size, so
     each matmul streams 50 columns = 2 consecutive timesteps:
     [K=21, M=50, N=50]; the two diagonal 25x25 blocks of the output are
     the useful products, off-diagonal blocks accumulate junk harmlessly.
     128 matmuls per batch accumulate into one [50, 50] PSUM tile.
     The emission is software-pipelined at matmul granularity: pair k+1's
     projection matmuls are interleaved into pair k's gram chain (one per
     ~19 quads) so the in-order PE sees a single dense stream - no stalls
     on the swap DMAs and no HAM clock re-throttle gaps.
  4. Extraction: ACT-copy [50,50] to SBUF bf16; two accumulating matmuls
     with block-selector stationaries [I;0], [0;I] fold the diagonal
     blocks exactly. Epilogue: tanh/alpha/A/softmax. Compute bf16, PSUM
     f32; tolerance 2e-2.
"""

import numpy as np
import ml_dtypes

N, C, T, V = 128, 64, 256, 25
IC = 21
NCORES = 8
NB = N // NCORES          # 16 batches per core
NPAIR = NB // 2           # 8
TV = T * V                # 6400
CHUNK = 512
NCHUNK = (TV + CHUNK - 1) // CHUNK
NTP = T // 2              # 128 gram matmuls per batch (2 t's each)


def _build_bass():
    import concourse.bass as bass
    import concourse.tile as tile
    from concourse import bacc, mybir


    f32 = mybir.dt.float32
    bf16 = mybir.dt.bfloat16

    nc = bacc.Bacc("TRN2", target_bir_lowering=False, debug=False,
                   num_devices=NCORES)

    x_ext = nc.dram_tensor("x", [NB, C, TV], bf16, kind="ExternalInput")
    wqk_ext = nc.dram_tensor("wqk", [2 * C, 128], bf16, kind="ExternalInput")
    bias_ext = nc.dram_tensor("bias", [128, 1], f32, kind="ExternalInput")
    emat_ext = nc.dram_tensor("emat", [2 * V, 2 * V], bf16,
                              kind="ExternalInput")
    a_ext = nc.dram_tensor("amat", [V, V], f32, kind="ExternalInput")
    alpha_ext = nc.dram_tensor("alpha", [1, 1], f32, kind="ExternalInput")
    out_ext = nc.dram_tensor("out", [NB, V, V], f32, kind="ExternalOutput")

    with tile.TileContext(nc) as tc:
        with (
            tc.tile_pool(name="singles", bufs=1) as singles,
            tc.tile_pool(name="xb", bufs=3) as xb_pool,
            tc.tile_pool(name="yz", bufs=3) as yz_pool,
            tc.tile_pool(name="sh", bufs=3) as sh_pool,
            tc.tile_pool(name="gs", bufs=3) as gs_pool,
            tc.tile_pool(name="zp", bufs=4, space="PSUM") as zp_pool,
            tc.tile_pool(name="gp", bufs=2, space="PSUM") as gp_pool,
            tc.tile_pool(name="o2", bufs=2, space="PSUM") as o2_pool,
            tc.tile_pool(name="ep", bufs=3) as ep_pool,
        ):
            wqk = singles.tile([2 * C, 128], bf16)
            nc.sync.dma_start(out=wqk[:], in_=wqk_ext[:, :])
            bias = singles.tile([128, 1], f32)
            nc.sync.dma_start(out=bias[:], in_=bias_ext[:, :])
            emat = singles.tile([2 * V, 2 * V], bf16)
            nc.sync.dma_start(out=emat[:], in_=emat_ext[:, :])
            amat = singles.tile([V, V], f32)
            nc.sync.dma_start(out=amat[:], in_=a_ext[:, :])
            alpha_sb = singles.tile([V, 1], f32)
            nc.sync.dma_start(
                out=alpha_sb[:],
                in_=bass.AP(tensor=alpha_ext, offset=0, ap=[[0, V], [1, 1]]),
            )

            tiles = {}

            def project(pair):
                """x load + projection GEMMs + bias copies + kb swaps.
                Generator: yields after each PE matmul so the driver can
                interleave with the previous pair's gram chain."""
                n0, n1 = 2 * pair, 2 * pair + 1
                xb = xb_pool.tile([128, TV], bf16)
                nc.sync.dma_start(out=xb[0:C, :], in_=x_ext[n0])
                nc.gpsimd.dma_start(out=xb[C:128, :], in_=x_ext[n1])
                yz = yz_pool.tile([128, TV], bf16)
                sh = sh_pool.tile([128, TV], bf16)
                tiles[pair] = (yz, sh)
                for ck in range(NCHUNK):
                    c0 = ck * CHUNK
                    cw = min(CHUNK, TV - c0)
                    zp = zp_pool.tile([128, CHUNK], f32)
                    nc.tensor.matmul(
                        out=zp[:, 0:cw],
                        lhsT=wqk[:, :],
                        rhs=xb[:, c0:c0 + cw],
                        start=True, stop=True,
                    )
                    # split the copy across ACT and DVE: halves the
                    # latency gating the swap DMAs below
                    hw_ = cw // 2
                    nc.scalar.activation(
                        out=yz[:, c0:c0 + hw_],
                        in_=zp[:, 0:hw_],
                        func=mybir.ActivationFunctionType.Identity,
                        bias=bias[:],
                        scale=1.0,
                    )
                    nc.vector.tensor_scalar(
                        out=yz[:, c0 + hw_:c0 + cw],
                        in0=zp[:, hw_:cw],
                        scalar1=bias[:, 0:1], scalar2=None,
                        op0=mybir.AluOpType.add,
                    )
                    # swap-copy every 2 chunks: halves sequencer issues
                    if ck % 2 == 1 or ck == NCHUNK - 1:
                        s0 = (ck - 1) * CHUNK if ck % 2 == 1 else ck * CHUNK
                        send = c0 + cw
                        nc.sync.dma_start(
                            out=sh[0:IC, s0:send],
                            in_=yz[32:32 + IC, s0:send])
                        nc.gpsimd.dma_start(
                            out=sh[64:64 + IC, s0:send],
                            in_=yz[96:96 + IC, s0:send])
                    yield

            def gram_and_epilogue(pair):
                """Generator: yields after each gram PE matmul."""
                yz, sh = tiles.pop(pair)
                n0, n1 = 2 * pair, 2 * pair + 1
                for h, n in ((0, n0), (1, n1)):
                    rb = 64 * h
                    gp = gp_pool.tile([2 * V, 2 * V], f32)
                    for tp in range(NTP):
                        f0 = tp * 2 * V
                        nc.tensor.matmul(
                            out=gp[:],
                            lhsT=yz[rb:rb + IC, f0:f0 + 2 * V],
                            rhs=sh[rb:rb + IC, f0:f0 + 2 * V],
                            start=(tp == 0), stop=(tp == NTP - 1),
                            skip_group_check=True,
                        )
                        yield
                    gs = gs_pool.tile([2 * V, 2 * V], bf16)
                    nc.scalar.activation(
                        out=gs[:], in_=gp[:],
                        func=mybir.ActivationFunctionType.Copy,
                    )
                    o2 = o2_pool.tile([V, V], f32)
                    nc.tensor.matmul(
                        out=o2[:], lhsT=emat[:, 0:V], rhs=gs[:, 0:V],
                        start=True, stop=False, skip_group_check=True,
                    )
                    nc.tensor.matmul(
                        out=o2[:], lhsT=emat[:, V:2 * V], rhs=gs[:, V:2 * V],
                        start=False, stop=True, skip_group_check=True,
                    )
                    th = ep_pool.tile([V, V], f32, tag="th")
                    nc.scalar.activation(
                        out=th[:], in_=o2[:],
                        func=mybir.ActivationFunctionType.Tanh,
                        scale=1.0 / float(IC * T),
                    )
                    sm = ep_pool.tile([V, V], f32, tag="sm")
                    nc.vector.scalar_tensor_tensor(
                        out=sm[:], in0=th[:], scalar=alpha_sb[:, 0:1],
                        in1=amat[:],
                        op0=mybir.AluOpType.mult, op1=mybir.AluOpType.add,
                    )
                    negmax = ep_pool.tile([V, 1], f32, tag="negmax")
                    nc.vector.tensor_reduce(
                        out=negmax[:], in_=sm[:],
                        axis=mybir.AxisListType.X, op=mybir.AluOpType.max,
                        negate=True,
                    )
                    esum = ep_pool.tile([V, 1], f32, tag="esum")
                    ex = ep_pool.tile([V, V], f32, tag="ex")
                    nc.scalar.activation(
                        out=ex[:], in_=sm[:],
                        func=mybir.ActivationFunctionType.Exp,
                        bias=negmax[:, 0:1], scale=1.0,
                        accum_out=esum[:, 0:1],
                    )
                    rcp = ep_pool.tile([V, 1], f32, tag="rcp")
                    nc.vector.reciprocal(out=rcp[:], in_=esum[:])
                    res = ep_pool.tile([V, V], f32, tag="res")
                    nc.vector.tensor_scalar(
                        out=res[:], in0=ex[:], scalar1=rcp[:, 0:1],
                        scalar2=None, op0=mybir.AluOpType.mult,
                    )
                    nc.gpsimd.dma_start(out=out_ext[n], in_=res[:])

            # Software pipeline: pair k+1's projection matmuls are
            # interleaved INTO pair k's gram chain (1 proj per ~19 quads)
            # so the in-order PE sees one continuous dense stream - no
            # stalls on the swap DMAs and no HAM re-throttle gaps.
            for _ in project(0):
                pass
            for pair in range(NPAIR):
                pg = project(pair + 1) if pair + 1 < NPAIR else None
                stride = max(1, (2 * NTP) // (NCHUNK + 1))
                cnt = 0
                for _ in gram_and_epilogue(pair):
                    cnt += 1
                    if pg is not None and cnt % stride == 0:
                        next(pg, None)
                if pg is not None:
                    for _ in pg:
                        pass

    nc.compile()
    return nc


_NC_CACHE = None


def _make_in_maps(inputs):
    x = np.asarray(inputs["x"], dtype=np.float32)
    Wa = np.asarray(inputs["Wa"], dtype=np.float32)
    Wb = np.asarray(inputs["Wb"], dtype=np.float32)
    ba = np.asarray(inputs["ba"], dtype=np.float32)
    bb = np.asarray(inputs["bb"], dtype=np.float32)
    A = np.asarray(inputs["A"], dtype=np.float32)
    alpha = np.asarray(inputs["alpha"], dtype=np.float32)

    # Stationary [128, 128]: col j -> psum partition j of the projection.
    wqk = np.zeros((2 * C, 128), dtype=np.float32)
    wqk[0:C, 0:IC] = Wa.T
    wqk[0:C, 32:32 + IC] = Wb.T
    wqk[C:2 * C, 64:64 + IC] = Wa.T
    wqk[C:2 * C, 96:96 + IC] = Wb.T
    wqk = wqk.astype(ml_dtypes.bfloat16)

    bias = np.zeros((128, 1), dtype=np.float32)
    bias[0:IC, 0] = ba
    bias[32:32 + IC, 0] = bb
    bias[64:64 + IC, 0] = ba
    bias[96:96 + IC, 0] = bb

    # emat cols 0-24: [I; 0] selects the t0 diag rows; cols 25-49: [0; I]
    emat = np.zeros((2 * V, 2 * V), dtype=np.float32)
    emat[0:V, 0:V] = np.eye(V)
    emat[V:2 * V, V:2 * V] = np.eye(V)
    emat = emat.astype(ml_dtypes.bfloat16)

    amat = np.ascontiguousarray(A.reshape(V, V))
    alpha_in = np.ascontiguousarray(alpha.reshape(1, 1))

    xbf = np.ascontiguousarray(x.reshape(NCORES, NB, C, TV)).astype(
        ml_dtypes.bfloat16)
    return [
        {"x": xbf[i], "wqk": wqk, "bias": bias, "emat": emat,
         "amat": amat, "alpha": alpha_in}
        for i in range(NCORES)
    ]


def kernel(x, Wa, ba, Wb, bb, A, alpha):
    from concourse.bass_utils import run_bass_kernel_spmd

    global _NC_CACHE
    if _NC_CACHE is None:
        _NC_CACHE = _build_bass()
    nc = _NC_CACHE

    in_maps = _make_in_maps(dict(x=x, Wa=Wa, ba=ba, Wb=Wb, bb=bb, A=A,
                                 alpha=alpha))
    res = run_bass_kernel_spmd(nc, in_maps, core_ids=list(range(NCORES)))
    out = np.concatenate([r["out"] for r in res.results], axis=0)
    return out.reshape(N, V, V)


# revision 38
# speedup vs baseline: 1.3271x; 1.0280x over previous
"""Adaptive graph construction kernel for 8 TRN2 NeuronCores.

Reference computation (per batch item n of 128):
    qa = Wa @ x[n] + ba        # [21, 256*25]   (x[n] viewed as [64, 256*25])
    kb = Wb @ x[n] + bb        # [21, 256*25]
    B[v, w] = sum_{i, t} qa[i, (t, v)] * kb[i, (t, w)]     # [25, 25]
    out[n]  = softmax(A + tanh(B / (21*256)) * alpha, axis=-1)

Strategy: pure data parallelism - batch N=128 sharded across 8 cores, no
collectives. Per core, batches processed in pairs:
  1. Projection: both batches' x stacked on 128 partitions, one GEMM per
     512-chunk with a 4-block stationary [128, 128] -> psum partitions
     0-20: qa even, 32-52: kb even, 64-84: qa odd, 96-116: kb odd.
     Bias-add + bf16 cast fused into the PSUM->SBUF copy (alternating
     ScalarE/VectorE so neither engine bottlenecks).
  2. Row-class swap DMAs (per 2 chunks) put kb at qa's base partitions
     (the PE needs equal operand bases); chunk granularity keeps them off
     the critical path.
  3. Gram: PE matmul overhead is ~60 cycles regardless of free size, so
     each matmul streams 50 columns = 2 consecutive timesteps:
     [K=21, M=50, N=50]; the two diagonal 25x25 blocks of the output are
     the useful products, off-diagonal blocks accumulate junk harmlessly.
     128 matmuls per batch accumulate into one [50, 50] PSUM tile; the
     even/odd batch chains interleave (PE row-groups 0/64) so each
     LDWEIGHTS overlaps the other chain's in-flight MATMUL.
     The emission is also software-pipelined at matmul granularity: pair
     k+1's projection matmuls are interleaved into pair k's gram chain
     (one per ~19 quads) so the in-order PE sees a single dense stream -
     no stalls on the swap DMAs and no HAM clock re-throttle gaps.
  4. Extraction: ACT-copy [50,50] to SBUF bf16; two accumulating matmuls
     with block-selector stationaries [I;0], [0;I] fold the diagonal
     blocks exactly. Epilogue: tanh/alpha/A/softmax. Compute bf16, PSUM
     f32; tolerance 2e-2.
"""

import numpy as np
import ml_dtypes

N, C, T, V = 128, 64, 256, 25
IC = 21
NCORES = 8
NB = N // NCORES          # 16 batches per core
NPAIR = NB // 2           # 8
TV = T * V                # 6400
CHUNK = 512
NCHUNK = (TV + CHUNK - 1) // CHUNK
NTP = T // 2              # 128 gram matmuls per batch (2 t's each)


def _build_bass():
    import concourse.bass as bass
    import concourse.tile as tile
    from concourse import bacc, mybir


    f32 = mybir.dt.float32
    bf16 = mybir.dt.bfloat16

    nc = bacc.Bacc("TRN2", target_bir_lowering=False, debug=False,
                   num_devices=NCORES)

    x_ext = nc.dram_tensor("x", [NB, C, TV], bf16, kind="ExternalInput")
    wqk_ext = nc.dram_tensor("wqk", [2 * C, 128], bf16, kind="ExternalInput")
    bias_ext = nc.dram_tensor("bias", [128, 1], f32, kind="ExternalInput")
    emat_ext = nc.dram_tensor("emat", [2 * V, 2 * V], bf16,
                              kind="ExternalInput")
    a_ext = nc.dram_tensor("amat", [V, V], f32, kind="ExternalInput")
    alpha_ext = nc.dram_tensor("alpha", [1, 1], f32, kind="ExternalInput")
    out_ext = nc.dram_tensor("out", [NB, V, V], f32, kind="ExternalOutput")

    with tile.TileContext(nc) as tc:
        with (
            tc.tile_pool(name="singles", bufs=1) as singles,
            tc.tile_pool(name="xb", bufs=3) as xb_pool,
            tc.tile_pool(name="yz", bufs=3) as yz_pool,
            tc.tile_pool(name="sh", bufs=3) as sh_pool,
            tc.tile_pool(name="gs", bufs=3) as gs_pool,
            tc.tile_pool(name="zp", bufs=4, space="PSUM") as zp_pool,
            tc.tile_pool(name="gp", bufs=1, space="PSUM") as gp_pool,
            tc.tile_pool(name="o2", bufs=2, space="PSUM") as o2_pool,
            tc.tile_pool(name="ep", bufs=3) as ep_pool,
        ):
            wqk = singles.tile([2 * C, 128], bf16)
            nc.sync.dma_start(out=wqk[:], in_=wqk_ext[:, :])
            bias = singles.tile([128, 1], f32)
            nc.sync.dma_start(out=bias[:], in_=bias_ext[:, :])
            emat = singles.tile([2 * V, 2 * V], bf16)
            nc.sync.dma_start(out=emat[:], in_=emat_ext[:, :])
            amat = singles.tile([V, V], f32)
            nc.sync.dma_start(out=amat[:], in_=a_ext[:, :])
            alpha_sb = singles.tile([V, 1], f32)
            nc.sync.dma_start(
                out=alpha_sb[:],
                in_=bass.AP(tensor=alpha_ext, offset=0, ap=[[0, V], [1, 1]]),
            )

            tiles = {}

            def project(pair):
                """x load + projection GEMMs + bias copies + kb swaps.
                Generator: yields after each PE matmul so the driver can
                interleave with the previous pair's gram chain."""
                n0, n1 = 2 * pair, 2 * pair + 1
                xb = xb_pool.tile([128, TV], bf16)
                nc.sync.dma_start(out=xb[0:C, :], in_=x_ext[n0])
                nc.gpsimd.dma_start(out=xb[C:128, :], in_=x_ext[n1])
                yz = yz_pool.tile([128, TV], bf16)
                sh = sh_pool.tile([128, TV], bf16)
                tiles[pair] = (yz, sh)
                for ck in range(NCHUNK):
                    c0 = ck * CHUNK
                    cw = min(CHUNK, TV - c0)
                    zp = zp_pool.tile([128, CHUNK], f32)
                    nc.tensor.matmul(
                        out=zp[:, 0:cw],
                        lhsT=wqk[:, :],
                        rhs=xb[:, c0:c0 + cw],
                        start=True, stop=True,
                    )
                    # split the copy across ACT and DVE: halves the
                    # latency gating the swap DMAs below
                    hw_ = cw // 2
                    nc.scalar.activation(
                        out=yz[:, c0:c0 + hw_],
                        in_=zp[:, 0:hw_],
                        func=mybir.ActivationFunctionType.Identity,
                        bias=bias[:],
                        scale=1.0,
                    )
                    nc.vector.tensor_scalar(
                        out=yz[:, c0 + hw_:c0 + cw],
                        in0=zp[:, hw_:cw],
                        scalar1=bias[:, 0:1], scalar2=None,
                        op0=mybir.AluOpType.add,
                    )
                    # swap-copy every 2 chunks: halves sequencer issues
                    if ck % 2 == 1 or ck == NCHUNK - 1:
                        s0 = (ck - 1) * CHUNK if ck % 2 == 1 else ck * CHUNK
                        send = c0 + cw
                        nc.sync.dma_start(
                            out=sh[0:IC, s0:send],
                            in_=yz[32:32 + IC, s0:send])
                        nc.gpsimd.dma_start(
                            out=sh[64:64 + IC, s0:send],
                            in_=yz[96:96 + IC, s0:send])
                    yield

            def gram_and_epilogue(pair):
                """Generator: yields after each gram PE matmul. The even
                and odd chains interleave (PE row-groups 0/64) so each
                LDWEIGHTS overlaps the other chain's MATMUL."""
                yz, sh = tiles.pop(pair)
                n0, n1 = 2 * pair, 2 * pair + 1
                gpe = gp_pool.tile([2 * V, 2 * V], f32, tag="gpe")
                gpo = gp_pool.tile([2 * V, 2 * V], f32, tag="gpo")
                gps = {0: gpe, 1: gpo}
                for tp in range(NTP):
                    f0 = tp * 2 * V
                    for h in (0, 1):
                        rb = 64 * h
                        nc.tensor.matmul(
                            out=gps[h][:],
                            lhsT=yz[rb:rb + IC, f0:f0 + 2 * V],
                            rhs=sh[rb:rb + IC, f0:f0 + 2 * V],
                            start=(tp == 0), stop=(tp == NTP - 1),
                            skip_group_check=True,
                        )
                        yield
                for h, n in ((0, n0), (1, n1)):
                    gp = gps[h]
                    gs = gs_pool.tile([2 * V, 2 * V], bf16)
                    nc.scalar.activation(
                        out=gs[:], in_=gp[:],
                        func=mybir.ActivationFunctionType.Copy,
                    )
                    o2 = o2_pool.tile([V, V], f32)
                    nc.tensor.matmul(
                        out=o2[:], lhsT=emat[:, 0:V], rhs=gs[:, 0:V],
                        start=True, stop=False, skip_group_check=True,
                    )
                    nc.tensor.matmul(
                        out=o2[:], lhsT=emat[:, V:2 * V], rhs=gs[:, V:2 * V],
                        start=False, stop=True, skip_group_check=True,
                    )
                    th = ep_pool.tile([V, V], f32, tag="th")
                    nc.scalar.activation(
                        out=th[:], in_=o2[:],
                        func=mybir.ActivationFunctionType.Tanh,
                        scale=1.0 / float(IC * T),
                    )
                    sm = ep_pool.tile([V, V], f32, tag="sm")
                    nc.vector.scalar_tensor_tensor(
                        out=sm[:], in0=th[:], scalar=alpha_sb[:, 0:1],
                        in1=amat[:],
                        op0=mybir.AluOpType.mult, op1=mybir.AluOpType.add,
                    )
                    negmax = ep_pool.tile([V, 1], f32, tag="negmax")
                    nc.vector.tensor_reduce(
                        out=negmax[:], in_=sm[:],
                        axis=mybir.AxisListType.X, op=mybir.AluOpType.max,
                        negate=True,
                    )
                    esum = ep_pool.tile([V, 1], f32, tag="esum")
                    ex = ep_pool.tile([V, V], f32, tag="ex")
                    nc.scalar.activation(
                        out=ex[:], in_=sm[:],
                        func=mybir.ActivationFunctionType.Exp,
                        bias=negmax[:, 0:1], scale=1.0,
                        accum_out=esum[:, 0:1],
                    )
                    rcp = ep_pool.tile([V, 1], f32, tag="rcp")
                    nc.vector.reciprocal(out=rcp[:], in_=esum[:])
                    res = ep_pool.tile([V, V], f32, tag="res")
                    nc.vector.tensor_scalar(
                        out=res[:], in0=ex[:], scalar1=rcp[:, 0:1],
                        scalar2=None, op0=mybir.AluOpType.mult,
                    )
                    nc.gpsimd.dma_start(out=out_ext[n], in_=res[:])

            # Software pipeline: pair k+1's projection matmuls are
            # interleaved INTO pair k's gram chain (1 proj per ~19 quads)
            # so the in-order PE sees one continuous dense stream - no
            # stalls on the swap DMAs and no HAM re-throttle gaps.
            for _ in project(0):
                pass
            for pair in range(NPAIR):
                pg = project(pair + 1) if pair + 1 < NPAIR else None
                stride = max(1, (2 * NTP) // (NCHUNK + 1))
                cnt = 0
                for _ in gram_and_epilogue(pair):
                    cnt += 1
                    if pg is not None and cnt % stride == 0:
                        next(pg, None)
                if pg is not None:
                    for _ in pg:
                        pass

    nc.compile()
    return nc


_NC_CACHE = None


def _make_in_maps(inputs):
    x = np.asarray(inputs["x"], dtype=np.float32)
    Wa = np.asarray(inputs["Wa"], dtype=np.float32)
    Wb = np.asarray(inputs["Wb"], dtype=np.float32)
    ba = np.asarray(inputs["ba"], dtype=np.float32)
    bb = np.asarray(inputs["bb"], dtype=np.float32)
    A = np.asarray(inputs["A"], dtype=np.float32)
    alpha = np.asarray(inputs["alpha"], dtype=np.float32)

    # Stationary [128, 128]: col j -> psum partition j of the projection.
    wqk = np.zeros((2 * C, 128), dtype=np.float32)
    wqk[0:C, 0:IC] = Wa.T
    wqk[0:C, 32:32 + IC] = Wb.T
    wqk[C:2 * C, 64:64 + IC] = Wa.T
    wqk[C:2 * C, 96:96 + IC] = Wb.T
    wqk = wqk.astype(ml_dtypes.bfloat16)

    bias = np.zeros((128, 1), dtype=np.float32)
    bias[0:IC, 0] = ba
    bias[32:32 + IC, 0] = bb
    bias[64:64 + IC, 0] = ba
    bias[96:96 + IC, 0] = bb

    # emat cols 0-24: [I; 0] selects the t0 diag rows; cols 25-49: [0; I]
    emat = np.zeros((2 * V, 2 * V), dtype=np.float32)
    emat[0:V, 0:V] = np.eye(V)
    emat[V:2 * V, V:2 * V] = np.eye(V)
    emat = emat.astype(ml_dtypes.bfloat16)

    amat = np.ascontiguousarray(A.reshape(V, V))
    alpha_in = np.ascontiguousarray(alpha.reshape(1, 1))

    xbf = np.ascontiguousarray(x.reshape(NCORES, NB, C, TV)).astype(
        ml_dtypes.bfloat16)
    return [
        {"x": xbf[i], "wqk": wqk, "bias": bias, "emat": emat,
         "amat": amat, "alpha": alpha_in}
        for i in range(NCORES)
    ]


def kernel(x, Wa, ba, Wb, bb, A, alpha):
    from concourse.bass_utils import run_bass_kernel_spmd

    global _NC_CACHE
    if _NC_CACHE is None:
        _NC_CACHE = _build_bass()
    nc = _NC_CACHE

    in_maps = _make_in_maps(dict(x=x, Wa=Wa, ba=ba, Wb=Wb, bb=bb, A=A,
                                 alpha=alpha))
    res = run_bass_kernel_spmd(nc, in_maps, core_ids=list(range(NCORES)))
    out = np.concatenate([r["out"] for r in res.results], axis=0)
    return out.reshape(N, V, V)
